# revision 45
# baseline (speedup 1.0000x reference)
"""Chunked attention Trainium2 Bass kernel (v14; 849us -> 564us).

v14: QK self-pairing -- each k-tile's 512 q cols run as two concurrent
256-col matmuls on opposite PE row halves, with a skewed score layout
(flat = 256*t + 256*nt*h) that places a tile's two halves in different
PSUM banks; exp reads the skew back with one packed (h t q) rearrange.
QK per group 600 -> 465ns; PE busy 519 -> 499us.

Problem: B=2, S=8192, HIDDEN=1024, HEADS=16, HEAD_DIM=64, CHUNK=2048,
OVERLAP=128. Sharding: head-parallel x batch-parallel -> 32 (b,h) jobs,
4 per core on 8 cores. Each core computes full-seq chunked attention for
its 4 heads; the host slices/pre-transposes inputs and reassembles the
output.

Evolution from the profiled v4 baseline (849us):
  - All matmul operands fp16 (1 cyc/col PE streaming like bf16, but
    ~8x less input-rounding noise; quarter the input DMA bytes of fp32).
  - PV runs ONE Q-BLOCK BEHIND QK/exp (software pipeline): each QK+exp
    group emission is followed by PV groups of the previous q-block,
    whose probs are long ready. v4 serialized PV behind the same
    q-block's exp tail, idling the PE ~2-4us every q-block; the HAM
    clock gate then re-throttled the PE to 1.2 GHz once per q-block
    (62 oscillations/run, 43% of time at half clock; now <10).
  - exp split between ACT (~82%, activation Exp -> fp16 probs) and DVE
    (~18%, at most one group per q-block): DVE computes exp via a
    single fp16 bit-trick rail plus a quadratic completed-square
    mantissa correction (6 ops, all in DVE 2x/4x perf modes -- the v4
    cubic needed two 1x-mode scalar_tensor_tensor ops; measured 7.9us
    -> ~5.3us per 1536-col group):
      y   = int16(s*A16 + B16)            # fp16 bits of ~exp(s/8)
      m   = y & 1023                      # mantissa bits
      x   = m * 2^-10 + H2                # ts mult+add
      w   = x*x                           # tt
      G   = C2*w + K2                     # ts == quad fit of 2^x/(1+x)
      pT  = G * bitcast_fp16(y)           # tt
    The PSUM-reading y gets tc.high_priority() (it gates the score-
    buffer rotation the PE waits on).
  - The ones column of [V|1] is appended host-side (the on-device
    strided broadcast copy cost 3.1us of DVE per chunk).
  - Chunk inputs are DMA-prefetched one chunk ahead (3-deep ops pool)
    on the GpSimd queue, away from the per-q-block output DMAs.
  - Output staging copy on DVE, DMA from SBUF (PSUM is not DMA- or
    GpSimd-accessible).

Measured regressions kept OUT (each cost ~50us): 2 DVE chains per
q-block (exceeds the per-q-block DVE budget; PE stalls on late probs),
GpSimd offload of the final chain multiply (3.3us op + 2us DRAIN made
pT late), QK 2-group batching for cross-group pairing (the Tile list
scheduler reorders anyway; QK is gated by the 2-buffer score rotation,
so the 3rd tile of each group cannot pair -- 3 full PSUM score buffers
would need 9 banks, PSUM has 8).

Per-core dataflow:
  - Host supplies Q^T and K^T in [d, seq] fp16 layout, duplicated across
    both 64-partition halves so contraction-64 QK matmuls can be
    row-packed in pairs (two concurrent matmuls in the PE array).
  - QK^T: S^T[k,q] fp16 matmuls into PSUM groups of 3 banks (fp32).
  - exp: ACT / DVE split per k-tile group (whole groups preferred).
  - PV: lhsT=[V|1] fp16 (65 cols, stationary) accumulates [O^T; l] into
    one PSUM bank over all k-tiles of the chunk.
  - The UNNORMALIZED per-chunk [O^T; l] (65 rows) goes to DRAM; softmax
    division and the 128-wide overlap-band blending happen on the host.
"""

import sys

if '/opt/trn_rl_repo' not in sys.path:
    sys.path.insert(0, '/opt/trn_rl_repo')

import numpy as np

import concourse.bass as bass
import concourse.mybir as mybir
import concourse.tile as tile
from concourse.bass_utils import run_bass_kernel_spmd

F32 = mybir.dt.float32
F16 = mybir.dt.float16
I16 = mybir.dt.int16
EXP = mybir.ActivationFunctionType.Exp
MULT = mybir.AluOpType.mult
ADD = mybir.AluOpType.add
AND = mybir.AluOpType.bitwise_and

B, S, HIDDEN, HEADS, HD = 2, 8192, 1024, 16, 64
SCALE = 1.0 / 8.0  # 1/sqrt(64)
N_CORES = 8
JOBS = 4  # (b, h) pairs per core
# (q0, Lq, k0, Lk) per chunk; step=1920, overlap=128
CHUNKS = [
    (0, 2048, 0, 2176),
    (1920, 2048, 1792, 2304),
    (3840, 2048, 3712, 2304),
    (5760, 2048, 5632, 2304),
    (7680, 512, 7552, 640),
]
COLS = [0, 2048, 4096, 6144, 8192]  # chunk col offsets in the out buffer
SQ = 8704  # sum of chunk Lq
GROUP = 3  # k-tiles per S^T PSUM group (3 banks x2 bufs + opsum x2 = 8)

# fp16 bit-trick exp constants (see module docstring). The mantissa
# correction 2^x/(1+x) on [0,1) is a minimax QUADRATIC in completed-
# square form c2*(x + H2)^2 + K2 (max rel err ~6.4e-3 incl fp16
# rounding; only the DVE-share columns see it).
A16 = float(SCALE * np.log2(np.e) * 1024.0)
B16 = 15360.0
C2 = 0.22682217912648842
H2 = -0.4777030826812545
K2 = 0.9422425990826463
# ACT fraction of exp columns (rate balance: DVE 6-op chain ~3.5ns/col
# vs ACT ~1.0ns/col). At most ONE k-tile group per q-block goes to DVE,
# its QK is emitted first and its PV last, so the chain's serial
# latency hides under the q-block's ACT-group work.
ACT_FRAC = 0.775


def _legalize_waits(nc, max_waits=1):
    """walrus in this config rejects >1 sync-wait per instruction: hoist
    excess waits onto injected same-engine NoOps placed just before."""
    cnt = 0
    for f in nc.m.functions:
        for blk in f.blocks:
            il = blk.instructions
            if not any(
                i.sync_info is not None and i.sync_info.on_wait
                and len(i.sync_info.on_wait) > max_waits for i in il
            ):
                continue
            new = []
            for inst in il:
                si = inst.sync_info
                if si is not None and si.on_wait and len(si.on_wait) > max_waits:
                    waits = list(si.on_wait)
                    spill, keep = waits[:-max_waits], waits[-max_waits:]
                    for w in spill:
                        nop = mybir.InstNoOp(
                            name=f"I-wsplit-{cnt}", ins=[], outs=[])
                        cnt += 1
                        nop.engine = inst.engine
                        nop.sync_info = mybir.SyncInfo(on_wait=[w], on_update=[])
                        new.append(nop)
                    inst.sync_info = mybir.SyncInfo(
                        on_wait=keep, on_update=list(si.on_update or []))
                new.append(inst)
            blk.instructions = new
    return cnt


def _build_nc(reps=1):
    nc = bass.Bass()
    qt_in = nc.declare_dram_parameter("qt", [JOBS, 128, S], F16,
                                      isOutput=False)
    kt_in = nc.declare_dram_parameter("kt", [JOBS, 128, S], F16,
                                      isOutput=False)
    v_in = nc.declare_dram_parameter("v", [JOBS, S, HD + 1], F16,
                                     isOutput=False)
    out = nc.declare_dram_parameter("out", [JOBS, 65, SQ], F32, isOutput=True)

    # running column counters for the ACT/DVE exp split (whole-group
    # granularity: the DVE chain amortizes better on big blocks)
    split_state = {"act": 0, "tot": 0, "rot": 0}

    def qblock_dve_groups(ngroups, nk):
        """Pick at most ONE group of this q-block for the DVE exp path,
        keeping the global ACT column ratio near ACT_FRAC. (Two chains
        per q-block exceed the per-q-block DVE budget and stall the PE
        on late probs -- measured regression.)"""
        cols = nk * 512
        t = split_state["tot"] + cols
        split_state["tot"] = t
        picked = []
        a_cur = split_state["act"] + cols
        g = split_state["rot"] % ngroups
        gtiles = min(GROUP, nk - g * GROUP)
        a_dve = a_cur - gtiles * 512
        if abs(a_dve / t - ACT_FRAC) < abs(a_cur / t - ACT_FRAC):
            picked.append(g)
            a_cur = a_dve
            split_state["rot"] += 1
        split_state["act"] = a_cur
        return picked

    with tile.TileContext(nc) as tc:
        with (
            tc.tile_pool(name="ops", bufs=4) as ops,          # qT/kT/vW
            tc.tile_pool(name="probs", bufs=13) as probs,     # pT (2 qblocks)
            tc.tile_pool(name="rails", bufs=2) as rails,      # DVE chain tmp
            tc.tile_pool(name="opath", bufs=3) as opath,      # o_sb staging
            tc.tile_pool(name="spsum", bufs=2, space="PSUM") as spsum,
            tc.tile_pool(name="onepsum", bufs=2, space="PSUM") as onepsum,
        ):
            # warm up the ACT Exp spline table at t=0: walrus inserts the
            # ~2.7us ACT_TABLE_LOAD before the FIRST ACTIVATE, which
            # otherwise serializes behind the first QK group; a dummy
            # 1-col activation overlaps it with the input DMAs instead
            warm = rails.tile([128, 1], F32, tag="warm")
            nc.vector.memset(warm, 0.0)
            warm2 = rails.tile([128, 1], F32, tag="warm2")
            nc.scalar.activation(warm2, warm, EXP, scale=SCALE)

            # Pending-PV state for the one-q-block-behind pipeline.
            # pend = dict(j, ci, qb, nk, pTs, pv_order, vW, opsum, emitted)
            pend_box = [None]

            def emit_pv_group(pend):
                """Emit the next not-yet-emitted PV group of the pending
                q-block. Returns False when exhausted."""
                i_ord = pend["emitted"]
                if i_ord >= len(pend["pv_order"]):
                    return False
                g = pend["pv_order"][i_ord]
                nk = pend["nk"]
                last_ord = len(pend["pv_order"]) - 1
                kts = list(range(g * GROUP, min((g + 1) * GROUP, nk)))
                pT = pend["pTs"][g]
                vW = pend["vW"]
                if pend["opsum"] is None:
                    opsum = onepsum.tile([128, 512], F32, tag="opsum")
                    pend["opsum"] = opsum
                opsum = pend["opsum"]
                for i, kt in enumerate(kts):
                    nc.tensor.matmul(
                        opsum[0:65, :],
                        vW[:, kt * 65:(kt + 1) * 65],
                        pT[:, i * 512:(i + 1) * 512],
                        start=(i_ord == 0 and i == 0),
                        stop=(i_ord == last_ord and i == len(kts) - 1),
                        skip_group_check=True,
                    )
                pend["emitted"] = i_ord + 1
                return True

            def flush_pv(pend):
                """Emit all remaining PV groups + O staging/DMA."""
                if pend is None:
                    return
                while emit_pv_group(pend):
                    pass
                o_sb = opath.tile([65, 512], F32, tag="osb")
                nc.vector.tensor_copy(o_sb, pend["opsum"][0:65, :])
                c0 = COLS[pend["ci"]] + pend["qb"] * 512
                nc.sync.dma_start(
                    out=out[pend["j"], :, c0:c0 + 512], in_=o_sb)

            def stage_chunk(j, ci):
                """DMA a chunk's qT/kT/vW into the double-buffered ops
                pool."""
                q0, lq, k0, lk = CHUNKS[ci]
                nk = lk // 128
                # input DMAs ride the otherwise-idle GpSimd queue so the
                # per-q-block output DMAs on the sync queue never
                # head-of-line-block a chunk prefetch
                qT = ops.tile([128, lq], F16, tag="qT",
                              padded_shape=[128, 2048])
                kT = ops.tile([128, lk], F16, tag="kT",
                              padded_shape=[128, 2304])
                # split into halves: subtile deps let the first QK
                # groups start on partial arrival
                hq, hk = lq // 2, lk // 2
                # first halves of BOTH q and k before the second halves:
                # the first QK group needs qT[:, :256] + kT tile 0 only
                nc.gpsimd.dma_start(out=qT[:, 0:hq],
                                    in_=qt_in[j, :, q0:q0 + hq])
                nc.gpsimd.dma_start(out=kT[:, 0:hk],
                                    in_=kt_in[j, :, k0:k0 + hk])
                nc.gpsimd.dma_start(out=qT[:, hq:lq],
                                    in_=qt_in[j, :, q0 + hq:q0 + lq])
                nc.gpsimd.dma_start(out=kT[:, hk:lk],
                                    in_=kt_in[j, :, k0 + hk:k0 + lk])
                vW = ops.tile([128, nk * 65], F16, tag="vW",
                              padded_shape=[128, 18 * 65])
                # v arrives host-side as [..., 65] with a ones column
                # already appended (the former on-device strided
                # broadcast copy cost 3.1us of DVE per chunk)
                nc.gpsimd.dma_start(
                    out=vW.rearrange("p (t e) -> p t e", e=65),
                    in_=v_in[j, k0:k0 + lk, :].rearrange(
                        "(t p) e -> p t e", p=128),
                )
                return (qT, kT, vW)

            # flat (job, chunk) order for two-chunk-ahead DMA prefetch
            jcs = [(j, ci) for _ in range(reps) for j in range(JOBS)
                   for ci in range(len(CHUNKS))]
            staged = {0: stage_chunk(*jcs[0])}
            if len(jcs) > 1:
                staged[1] = stage_chunk(*jcs[1])
            for jci, (j, ci) in enumerate(jcs):
                if True:
                    q0, lq, k0, lk = CHUNKS[ci]
                    nk = lk // 128
                    qT, kT, vW = staged.pop(jci)

                    ngroups = (nk + GROUP - 1) // GROUP
                    for qb in range(lq // 512):
                        qs = slice(qb * 512, qb * 512 + 512)
                        dve_gs = qblock_dve_groups(ngroups, nk)
                        # group emission order: the DVE groups' QK goes
                        # FIRST (chain latency hides under the ACT
                        # groups' work) and their PV goes LAST.
                        order = dve_gs + [g for g in range(ngroups)
                                          if g not in dve_gs]
                        pTs = {}
                        ys = {}
                        pend = pend_box[0]
                        qk_par = 0
                        for gi, g in enumerate(order):
                            kts = list(range(
                                g * GROUP, min((g + 1) * GROUP, nk)))
                            nt = len(kts)
                            sp = spsum.tile([128, 512 * GROUP], F32,
                                            tag="sp")
                            # QK^T: S^T[k,q]; every k-tile SELF-PAIRS:
                            # its 512 q cols split into two 256-col
                            # matmuls on opposite PE row halves (q/k
                            # duplicated per half) that run concurrently.
                            # Skewed score layout flat = 256*t + 256*nt*h
                            # puts a tile's two halves in DIFFERENT PSUM
                            # banks (concurrent same-bank writes fault;
                            # start=True clears a whole bank). Only the
                            # first piece landing in each bank carries
                            # start=True; later pieces are >=256 cycles
                            # behind the clear, overwrite-where-unwritten.
                            banks_started = set()
                            for i, kt in enumerate(kts):
                                for hh in range(2):
                                    flat = 256 * i + 256 * nt * hh
                                    bank = flat // 512
                                    st = bank not in banks_started
                                    banks_started.add(bank)
                                    rows = slice(64 * hh, 64 * hh + 64)
                                    nc.tensor.matmul(
                                        sp[:, flat:flat + 256],
                                        kT[rows, kt * 128:(kt + 1) * 128],
                                        qT[rows,
                                           qs.start + 256 * hh:
                                           qs.start + 256 * hh + 256],
                                        start=st, stop=True,
                                        tile_position=(64 * hh, 0),
                                        skip_group_check=True,
                                    )
                            # the skew is exactly a packed (h t q)
                            # factorization: exp reads it with one
                            # regular AP, writing pT tile-major
                            spv = sp[:, 0:512 * nt].rearrange(
                                "p (h t q) -> p t h q", h=2, q=256)
                            sl = slice(0, 512 * len(kts))
                            if g not in dve_gs:
                                pT = probs.tile([128, 512 * GROUP], F16,
                                                tag="pT")
                                nc.scalar.activation(
                                    pT[:, sl], spv, EXP, scale=SCALE)
                                pTs[g] = pT
                            else:
                                # DVE path: only the PSUM-reading y here;
                                # it gates the score-buffer rotation (the
                                # PE's QK two groups later waits on it),
                                # so let it jump the DVE ready-queue
                                # ahead of older copies/chain ops.
                                #   y = int16(s*A16+B16) [1x: PSUM rd]
                                y = rails.tile([128, 512 * GROUP], I16,
                                               tag="y")
                                with tc.high_priority():
                                    nc.vector.tensor_scalar(
                                        y[:, sl], spv, A16, B16,
                                        MULT, ADD)
                                ys[g] = y
                            if gi == len(dve_gs) - 1:
                                # all dve ys emitted; now their chain
                                # tails:
                                #   m = y & 1023         [4x]
                                #   x = m/1024 + H2      [4x]
                                #   w = x*x              [2x]
                                #   G = C2*w + K2        [4x]
                                #   p = G * bitcast16(y) [2x]
                                for gd in dve_gs:
                                    kd = min(GROUP, nk - gd * GROUP)
                                    sl = slice(0, 512 * kd)
                                    y = ys[gd]
                                    m = rails.tile([128, 512 * GROUP], I16,
                                                   tag="m")
                                    nc.vector.tensor_scalar(
                                        m[:, sl], y[:, sl], 1023, None, AND)
                                    xv = rails.tile([128, 512 * GROUP], F16,
                                                    tag="xv")
                                    nc.vector.tensor_scalar(
                                        xv[:, sl], m[:, sl], 1.0 / 1024.0,
                                        H2, MULT, ADD)
                                    w = rails.tile([128, 512 * GROUP], F16,
                                                   tag="w")
                                    nc.vector.tensor_tensor(
                                        w[:, sl], xv[:, sl], xv[:, sl],
                                        MULT)
                                    gg = rails.tile([128, 512 * GROUP], F16,
                                                    tag="gg")
                                    nc.vector.tensor_scalar(
                                        gg[:, sl], w[:, sl], C2, K2,
                                        MULT, ADD)
                                    pT = probs.tile([128, 512 * GROUP], F16,
                                                    tag="pT")
                                    # (GpSimd offload of this multiply was
                                    # tried: its 3.3us op + 2us DRAIN made
                                    # pT late and stalled the PV matmuls)
                                    nc.vector.tensor_tensor(
                                        pT[:, sl], gg[:, sl],
                                        y[:, sl].bitcast(F16), MULT)
                                    pTs[gd] = pT
                            # interleave: PV groups of the PREVIOUS
                            # q-block (probs long ready) after every
                            # SECOND QK group. (Emitting them BEFORE the
                            # 2nd QK group in priority order -- to batch
                            # PV runs and halve QK<->PV row-config drains
                            # -- measured 569us -> 680us: starving the
                            # QK/exp front of priority inflates every
                            # engine's wait time.)
                            if pend is not None:
                                emit_pv_group(pend)
                        # finish the previous q-block (leftover PV groups,
                        # O copy-out + store DMA)
                        flush_pv(pend)
                        pv_order = ([g for g in range(ngroups)
                                     if g not in dve_gs] + dve_gs)
                        pend_box[0] = {
                            "j": j, "ci": ci, "qb": qb, "nk": nk,
                            "pTs": pTs, "pv_order": pv_order, "vW": vW,
                            "opsum": None, "emitted": 0,
                        }
                        if qb == 0 and jci + 2 < len(jcs):
                            # prefetch two chunks ahead (4-deep ops
                            # pool; the WAR on the previous user of the
                            # buffer is tracked by Tile)
                            staged[jci + 2] = stage_chunk(*jcs[jci + 2])
            flush_pv(pend_box[0])

    _legalize_waits(nc)
    return nc


_NC = None


def _get_nc():
    global _NC
    if _NC is None:
        _NC = _build_nc()
    return _NC


def make_in_maps(query, key_, value):
    """Host-side prep: per-core slices; Q^T/K^T in [d, seq] fp16 layout
    duplicated across both partition halves."""
    f16 = np.float16
    qh = query.reshape(B, S, HEADS, HD)
    kh = key_.reshape(B, S, HEADS, HD)
    vh = value.reshape(B, S, HEADS, HD).astype(f16)
    qT = np.ascontiguousarray(qh.transpose(0, 2, 3, 1)).astype(f16)
    kT = np.ascontiguousarray(kh.transpose(0, 2, 3, 1)).astype(f16)
    in_maps = []
    for c in range(N_CORES):
        jobs = [(g // HEADS, g % HEADS) for g in range(4 * c, 4 * c + 4)]
        qt_c = np.empty((JOBS, 128, S), f16)
        kt_c = np.empty((JOBS, 128, S), f16)
        v_c = np.empty((JOBS, S, HD + 1), f16)
        v_c[:, :, HD] = 1.0  # ones column for the l row of [O^T; l]
        for jj, (b, h) in enumerate(jobs):
            qt_c[jj, 0:64] = qT[b, h]
            qt_c[jj, 64:128] = qT[b, h]
            kt_c[jj, 0:64] = kT[b, h]
            kt_c[jj, 64:128] = kT[b, h]
            v_c[jj, :, 0:HD] = vh[b, :, h]
        in_maps.append({"qt": qt_c, "kt": kt_c, "v": v_c})
    return in_maps


def assemble_out(results):
    """Host: per-chunk softmax division + overlap-band blending (fp32,
    mirrors the reference's merge), then scatter into [B, S, HIDDEN]."""
    wt = np.linspace(1.0, 0.0, 128).astype(np.float32)  # prev-chunk tail
    wh = np.linspace(0.0, 1.0, 128).astype(np.float32)  # cur-chunk head
    denom = (wt + wh) + np.float32(1e-10)
    a = (wt / denom).astype(np.float32)[:, None]
    bb = (wh / denom).astype(np.float32)[:, None]

    out = np.empty((B, S, HIDDEN), dtype=np.float32)
    for c in range(N_CORES):
        oc = results[c]["out"]  # [4, 65, SQ]
        for jj, g in enumerate(range(4 * c, 4 * c + 4)):
            b, h = g // HEADS, g % HEADS
            full = np.empty((S, HD), np.float32)
            prev_tail = None
            for ci, (q0, lq, k0, lk) in enumerate(CHUNKS):
                off = COLS[ci]
                blk = oc[jj, :, off:off + lq]
                on = (blk[0:64] / blk[64:65]).T  # [lq, 64] normalized
                lo = 0
                if ci > 0:
                    full[q0:q0 + 128] = prev_tail * a + on[0:128] * bb
                    lo = 128
                hi = lq
                if ci < len(CHUNKS) - 1:
                    hi = lq - 128
                    prev_tail = on[lq - 128:lq]
                full[q0 + lo:q0 + hi] = on[lo:hi]
            out[b, :, h * HD:(h + 1) * HD] = full
    return out


def kernel(query, key, value):
    query = np.asarray(query, dtype=np.float32)
    key_ = np.asarray(key, dtype=np.float32)
    value = np.asarray(value, dtype=np.float32)
    nc = _get_nc()
    in_maps = make_in_maps(query, key_, value)
    res = run_bass_kernel_spmd(nc, in_maps, list(range(N_CORES)))
    return assemble_out(res.results)

